# revision 7
# baseline (speedup 1.0000x reference)
"""BiGCN (3-layer binary GCN) on 8 TRN2 NeuronCores.

Strategy (per sharding hint): partition nodes (rows) across 8 cores; each
core owns N/8 destination nodes and all edges pointing at them. Per layer:
  z = sign(h) @ Wb            (dense matmul on own nodes, fp16)
  AllGather z -> full table   (cross-core exchange)
  agg[dst] = sum_e norm_e * z[src_e]   (dma_gather rows + one-hot matmul
                                        accumulation in PSUM, sorted by dst)
  h_next = agg + b            (fused into Sign / epilogue)
Matmul and aggregation commute (aggregation is linear), so aggregating the
post-matmul activations minimizes both exchange and gather width.
Batchnorm: sign((x - mu)) — the variance cancels inside sign().

Host side does only index/degree preprocessing (sharding, edge sorting,
padding) and weight binarization; all O(N*F) float work runs on device.
"""

import math

import numpy as np

import concourse.bass as bass
import concourse.mybir as mybir
from concourse import bacc
from concourse.tile import TileContext
from concourse.bass_utils import run_bass_kernel_spmd
from concourse.masks import make_identity

P = 128

FULL_CFG = dict(
    N=100000,
    IN=256,
    H=128,
    OUT=40,
    NCORES=8,
    NCH=4,  # src chunks for int16 gather indices
)


def _prep(x, edge_index, W0, b0, W1, b1, W2, b2, cfg):
    """Host-side sharding: degrees, edge bucketing by (core, dst-block,
    src-chunk), padding to a uniform tile structure, weight binarization.
    Returns (in_maps, meta)."""
    N = cfg["N"]
    IN = cfg["IN"]
    H = cfg["H"]
    OUT = cfg["OUT"]
    C = cfg["NCORES"]
    NCH = cfg["NCH"]
    SH = N // C
    NB = (SH + P - 1) // P
    CH = (N + NCH - 1) // NCH
    assert CH <= 32768, "gather indices are int16"
    assert N % C == 0

    x = np.asarray(x, dtype=np.float32)
    ei = np.asarray(edge_index)
    src = ei[0].astype(np.int64)
    dst = ei[1].astype(np.int64)

    deg = np.bincount(dst, minlength=N).astype(np.float64) + 1.0
    dinv = (deg ** -0.5).astype(np.float32)

    loop = np.arange(N, dtype=np.int64)
    srcA = np.concatenate([src, loop])
    dstA = np.concatenate([dst, loop])
    wA = (dinv[srcA] * dinv[dstA]).astype(np.float32)

    core = dstA // SH
    rel = dstA - core * SH
    blk = rel // P
    dloc = (rel - blk * P).astype(np.float32)
    q = srcA // CH
    il = (srcA - q * CH).astype(np.int16)

    key = (core * NB + blk) * NCH + q
    order = np.argsort(key, kind="stable")
    key_s = key[order]
    il_s = il[order]
    dloc_s = dloc[order]
    w_s = wA[order]

    NGRP = C * NB * NCH
    counts = np.bincount(key_s, minlength=NGRP)
    KT = int(math.ceil(counts.max() / P))
    CAP = KT * P

    # position of each edge within its group
    starts = np.zeros(NGRP + 1, dtype=np.int64)
    np.cumsum(counts, out=starts[1:])
    pos = np.arange(len(key_s)) - starts[key_s]

    GRP = NB * NCH  # groups per core
    A_idx = np.zeros((C, GRP, CAP), dtype=np.int16)
    A_dl = np.zeros((C, GRP, CAP), dtype=np.float32)
    A_w = np.zeros((C, GRP, CAP), dtype=np.float32)
    ccore = key_s // (NB * NCH)
    cgrp = key_s - ccore * (NB * NCH)
    A_idx[ccore, cgrp, pos] = il_s
    A_dl[ccore, cgrp, pos] = dloc_s
    A_w[ccore, cgrp, pos] = w_s

    # wrapped int16 index layout: edge j of a group -> [j%16, j//16],
    # replicated to 128 partitions (8 copies) for the 8 SWDGE Q7 cores.
    idx_w = (
        A_idx.reshape(C, GRP, CAP // 16, 16)
        .transpose(0, 3, 1, 2)
        .reshape(C, 16, GRP * (CAP // 16))
    )
    idx_all = np.tile(idx_w, (1, 8, 1))  # [C, 128, IDXCOLS]
    # per-tile columns: tile t=(g*KT+s), partition p = edge s*128+p of group g
    dl_t = A_dl.reshape(C, GRP, KT, P).transpose(0, 3, 1, 2).reshape(C, P, GRP * KT)
    w_t = A_w.reshape(C, GRP, KT, P).transpose(0, 3, 1, 2).reshape(C, P, GRP * KT)

    # weight binarization (XNOR-style): sign(W) * mean(|W|, axis=0)
    def binw(W, pad_to=None):
        W = np.asarray(W, dtype=np.float32)
        alpha = np.abs(W).mean(axis=0, keepdims=True)
        Wb32 = np.sign(W) * alpha
        hi = Wb32.astype(np.float16)
        lo = (Wb32 - hi.astype(np.float32)).astype(np.float16)
        if pad_to is not None and hi.shape[1] < pad_to:
            hi = np.pad(hi, ((0, 0), (0, pad_to - hi.shape[1])))
            lo = np.pad(lo, ((0, 0), (0, pad_to - lo.shape[1])))
        return hi, lo

    Wb0, Wb0l = binw(W0)
    Wb1, Wb1l = binw(W1)
    Wb2, Wb2l = binw(W2, pad_to=P)
    bias0 = np.asarray(b0, np.float32).reshape(H, 1)
    bias1 = np.asarray(b1, np.float32).reshape(H, 1)
    bias2 = np.zeros((P, 1), np.float32)
    bias2[:OUT, 0] = np.asarray(b2, np.float32)

    iota_row = np.tile(np.arange(P, dtype=np.float32), (P, 1))

    in_maps = []
    for c in range(C):
        xs = np.ascontiguousarray(x[c * SH : (c + 1) * SH].T)  # [IN, SH] f32
        in_maps.append(
            dict(
                x_t=xs,
                Wb0=Wb0,
                Wb1=Wb1,
                Wb2=Wb2,
                Wb0l=Wb0l,
                Wb1l=Wb1l,
                Wb2l=Wb2l,
                bias0=bias0,
                bias1=bias1,
                bias2=bias2,
                idx_all=np.ascontiguousarray(idx_all[c]),
                dl_t=np.ascontiguousarray(dl_t[c]),
                w_t=np.ascontiguousarray(w_t[c]),
                iota_row=iota_row,
            )
        )
    meta = dict(KT=KT, SH=SH, NB=NB, CH=CH)
    return in_maps, meta


def _build(cfg, meta, dbg=False):
    """Build the SPMD Bass kernel (same program for all cores)."""
    N = cfg["N"]
    IN = cfg["IN"]
    H = cfg["H"]
    OUT = cfg["OUT"]
    C = cfg["NCORES"]
    NCH = cfg["NCH"]
    KT = meta["KT"]
    SH = meta["SH"]
    NB = meta["NB"]
    CH = meta["CH"]
    SHP = NB * P
    GRP = NB * NCH
    NT = GRP * KT
    IDXCOLS = GRP * KT * 8
    KC = (IN + P - 1) // P  # K-chunks for layer-0 matmul
    assert H == P
    f16 = mybir.dt.float16
    f32 = mybir.dt.float32
    AF = mybir.ActivationFunctionType
    OP = mybir.AluOpType
    rg = [list(range(C))]

    nc = bacc.Bacc(None, target_bir_lowering=False, debug=False)

    x_t = nc.dram_tensor("x_t", [IN, SH], f32, kind="ExternalInput")
    Wb0 = nc.dram_tensor("Wb0", [IN, H], f16, kind="ExternalInput")
    Wb1 = nc.dram_tensor("Wb1", [H, H], f16, kind="ExternalInput")
    Wb2 = nc.dram_tensor("Wb2", [H, P], f16, kind="ExternalInput")
    Wb0l = nc.dram_tensor("Wb0l", [IN, H], f16, kind="ExternalInput")
    Wb1l = nc.dram_tensor("Wb1l", [H, H], f16, kind="ExternalInput")
    Wb2l = nc.dram_tensor("Wb2l", [H, P], f16, kind="ExternalInput")
    bias0 = nc.dram_tensor("bias0", [H, 1], f32, kind="ExternalInput")
    bias1 = nc.dram_tensor("bias1", [H, 1], f32, kind="ExternalInput")
    bias2 = nc.dram_tensor("bias2", [P, 1], f32, kind="ExternalInput")
    idx_all = nc.dram_tensor("idx_all", [P, IDXCOLS], mybir.dt.int16, kind="ExternalInput")
    dl_td = nc.dram_tensor("dl_t", [P, NT], f32, kind="ExternalInput")
    w_td = nc.dram_tensor("w_t", [P, NT], f32, kind="ExternalInput")
    iota_d = nc.dram_tensor("iota_row", [P, P], f32, kind="ExternalInput")
    out_d = nc.dram_tensor("out", [SH, OUT], f32, kind="ExternalOutput")
    if dbg:
        dbg_z = nc.dram_tensor("dbg_z", [N, H], f32, kind="ExternalOutput")
        dbg_s = nc.dram_tensor("dbg_s", [IN, SH], f16, kind="ExternalOutput")
        dbg_a = nc.dram_tensor("dbg_a", [H, SH], f32, kind="ExternalOutput")

    with TileContext(nc) as tc:
        with (
            tc.tile_pool(name="persist", bufs=1) as pp,
            tc.tile_pool(name="xin", bufs=2) as xp,
            tc.tile_pool(name="gpool", bufs=3) as gp,
            tc.tile_pool(name="tpool", bufs=4) as tp,
            tc.tile_pool(name="zpool", bufs=3) as zp,
            tc.tile_pool(name="spool", bufs=2) as sp,
            tc.tile_pool(name="psz", bufs=2, space="PSUM") as psz,
            tc.tile_pool(name="psa", bufs=2, space="PSUM") as psa,
            tc.tile_pool(name="pst", bufs=2, space="PSUM") as pst,
            tc.tile_pool(name="dram", bufs=1, space="DRAM") as dp,
        ):
            # ---- persistent SBUF state ----
            idx_sb = pp.tile([P, IDXCOLS], mybir.dt.int16)
            dl_sb = pp.tile([P, NT], f32)
            w_sb = pp.tile([P, NT], f32)
            iota_sb = pp.tile([P, P], f32)
            wb_sb = [pp.tile([P, H], f16, name=f"wb0_{k}") for k in range(KC)]
            wb1_sb = pp.tile([P, H], f16)
            wb2_sb = pp.tile([P, P], f16)
            wbl_sb = [pp.tile([P, H], f16, name=f"wb0l_{k}") for k in range(KC)]
            wb1l_sb = pp.tile([P, H], f16)
            wb2l_sb = pp.tile([P, P], f16)
            b0_sb = pp.tile([H, 1], f32)
            b1_sb = pp.tile([H, 1], f32)
            b2_sb = pp.tile([P, 1], f32)
            # sign buffers: layer0 needs KC x [P, SHP]; s_buf[0] reused by L1/L2
            s_buf = [pp.tile([P, SHP], f16, name=f"sbuf_{k}") for k in range(KC)]
            nmu = [pp.tile([P, 1], f32, name=f"nmu_{k}") for k in range(KC)]

            nc.sync.dma_start(out=idx_sb[:], in_=idx_all[:])
            nc.sync.dma_start(out=dl_sb[:], in_=dl_td[:])
            nc.sync.dma_start(out=w_sb[:], in_=w_td[:])
            nc.sync.dma_start(out=iota_sb[:], in_=iota_d[:])
            for k in range(KC):
                nc.sync.dma_start(out=wb_sb[k][:], in_=Wb0[k * P : (k + 1) * P, :])
                nc.sync.dma_start(out=wbl_sb[k][:], in_=Wb0l[k * P : (k + 1) * P, :])
            nc.sync.dma_start(out=wb1_sb[:], in_=Wb1[:])
            nc.sync.dma_start(out=wb2_sb[:], in_=Wb2[:])
            nc.sync.dma_start(out=wb1l_sb[:], in_=Wb1l[:])
            nc.sync.dma_start(out=wb2l_sb[:], in_=Wb2l[:])
            nc.sync.dma_start(out=b0_sb[:], in_=bias0[:])
            nc.sync.dma_start(out=b1_sb[:], in_=bias1[:])
            nc.sync.dma_start(out=b2_sb[:], in_=bias2[:])

            # ---- DRAM bounce buffers ----
            bn_in = dp.tile([IN, 1], f32)
            bn_out = dp.tile([IN, 1], f32, addr_space="Shared")
            z_in = [dp.tile([SH, H], f32, name=f"z_in{k}") for k in range(3)]
            z_tab = [
                dp.tile([N, H], f32, name=f"z_tab{k}", addr_space="Shared")
                for k in range(3)
            ]

            # x segments
            nseg = max(1, (SH + 2559) // 2560)
            seg = (SH + nseg - 1) // nseg
            segs = [(i * seg, min(SH, (i + 1) * seg)) for i in range(nseg)]

            # ---- BatchNorm: global column means via AllReduce ----
            for k in range(KC):
                stats = sp.tile([P, nseg], f32, name="stats")
                for t, (a, b) in enumerate(segs):
                    xt = xp.tile([P, seg], f32, name="xt")
                    nc.sync.dma_start(out=xt[:, : b - a], in_=x_t[k * P : (k + 1) * P, a:b])
                    nc.vector.reduce_sum(
                        stats[:, t : t + 1], xt[:, : b - a], axis=mybir.AxisListType.X
                    )
                s1 = sp.tile([P, 1], f32, name="s1")
                nc.vector.reduce_sum(s1[:], stats[:], axis=mybir.AxisListType.X)
                nc.sync.dma_start(out=bn_in[k * P : (k + 1) * P, :], in_=s1[:])
            nc.gpsimd.collective_compute(
                "AllReduce", OP.add, replica_groups=rg,
                ins=[bn_in[:].opt()], outs=[bn_out[:].opt()],
            )
            for k in range(KC):
                s1g = sp.tile([P, 1], f32, name="s1g")
                nc.sync.dma_start(out=s1g[:], in_=bn_out[k * P : (k + 1) * P, :])
                nc.vector.tensor_scalar(
                    out=nmu[k][:], in0=s1g[:], scalar1=-1.0 / N, scalar2=None,
                    op0=OP.mult,
                )

            # ---- layer-0 input signs: sign(x - mu) ----
            for k in range(KC):
                for a, b in segs:
                    xt = xp.tile([P, seg], f32, name="xt2")
                    nc.sync.dma_start(out=xt[:, : b - a], in_=x_t[k * P : (k + 1) * P, a:b])
                    nc.scalar.activation(
                        out=s_buf[k][:, a:b], in_=xt[:, : b - a], func=AF.Sign,
                        bias=nmu[k][:, 0:1],
                    )
                if SHP > SH:
                    nc.gpsimd.memset(s_buf[k][:, SH:SHP], 1.0)

            if dbg:
                for k in range(KC):
                    nc.sync.dma_start(out=dbg_s[k * P : (k + 1) * P, :], in_=s_buf[k][:, :SH])

            ident = pp.tile([P, P], f32)
            make_identity(nc, ident[:])

            rows_last = SH - (NB - 1) * P

            # ---- the three GCN layers ----
            for layer in range(3):
                if layer == 0:
                    wmats = [(k, wb_sb[k]) for k in range(KC)] + [
                        (k, wbl_sb[k]) for k in range(KC)
                    ]
                elif layer == 1:
                    wmats = [(0, wb1_sb), (0, wb1l_sb)]
                else:
                    wmats = [(0, wb2_sb), (0, wb2l_sb)]
                bias_sb = (b0_sb, b1_sb, b2_sb)[layer]
                zi, zt = z_in[layer], z_tab[layer]

                # z = sign(h) @ Wb for own nodes
                for nt in range(NB):
                    pz = psz.tile([P, H if layer < 2 else P], f32, name="pz")
                    for i, (k, wm) in enumerate(wmats):
                        nc.tensor.matmul(
                            pz[:], lhsT=s_buf[k][:, nt * P : (nt + 1) * P], rhs=wm[:],
                            start=(i == 0), stop=(i == len(wmats) - 1),
                        )
                    zr = zp.tile([P, H], f32, name="zr")
                    nc.scalar.activation(out=zr[:], in_=pz[:, :H], func=AF.Copy)
                    rows = P if nt < NB - 1 else rows_last
                    nc.sync.dma_start(
                        out=zi[nt * P : nt * P + rows, :], in_=zr[:rows, :]
                    )

                nc.gpsimd.collective_compute(
                    "AllGather", OP.bypass, replica_groups=rg,
                    ins=[zi[:].opt()], outs=[zt[:].opt()],
                )

                if dbg and layer == 1:
                    nc.sync.dma_start(out=dbg_z[:], in_=zt[:])

                # aggregation over sorted edges, one dst-block at a time
                for b in range(NB):
                    pa = psa.tile([P, P], f32, name="pa")
                    for q in range(NCH):
                        g = b * NCH + q
                        G = gp.tile([P, KT, H], f32, name="G")
                        nc.gpsimd.dma_gather(
                            G[:],
                            zt[q * CH : min(N, (q + 1) * CH), :],
                            idx_sb[:, g * KT * 8 : (g + 1) * KT * 8],
                            num_idxs=KT * P,
                            num_idxs_reg=KT * P,
                            elem_size=H,
                            elem_step=H,
                        )
                        for s in range(KT):
                            tcol = g * KT + s
                            T = tp.tile([P, P], f32, name="T")
                            nc.vector.tensor_scalar(
                                out=T[:], in0=iota_sb[:],
                                scalar1=dl_sb[:, tcol : tcol + 1],
                                scalar2=w_sb[:, tcol : tcol + 1],
                                op0=OP.is_equal, op1=OP.mult,
                            )
                            nc.tensor.matmul(
                                pa[:], lhsT=G[:, s, :], rhs=T[:],
                                start=(q == 0 and s == 0),
                                stop=(q == NCH - 1 and s == KT - 1),
                            )
                    bs = b * P
                    rows = P if b < NB - 1 else rows_last
                    if layer < 2:
                        if dbg and layer == 1:
                            hh = sp.tile([P, P], f32, name="hh")
                            nc.scalar.activation(
                                out=hh[:], in_=pa[:], func=AF.Identity, bias=bias_sb[:, 0:1]
                            )
                            nc.sync.dma_start(out=dbg_a[:, bs : bs + rows], in_=hh[:, :rows])
                        nc.scalar.activation(
                            out=s_buf[0][:, bs : bs + P], in_=pa[:], func=AF.Sign,
                            bias=bias_sb[:, 0:1],
                        )
                    else:
                        h2 = sp.tile([P, P], f32, name="h2")
                        nc.scalar.activation(
                            out=h2[:], in_=pa[:], func=AF.Identity, bias=bias_sb[:, 0:1]
                        )
                        ls_ps = pst.tile([P, P], f32, name="ls_ps")
                        nc.tensor.transpose(ls_ps[:], h2[:], ident[:])
                        ls = sp.tile([P, P], f32, name="ls")
                        nc.scalar.activation(out=ls[:], in_=ls_ps[:], func=AF.Copy)
                        m = sp.tile([P, 1], f32, name="m")
                        nc.vector.reduce_max(m[:], ls[:, :OUT], axis=mybir.AxisListType.X)
                        sub = sp.tile([P, OUT], f32, name="sub")
                        nc.vector.tensor_scalar(
                            out=sub[:], in0=ls[:, :OUT], scalar1=m[:, 0:1], scalar2=None,
                            op0=OP.subtract,
                        )
                        ex = sp.tile([P, OUT], f32, name="ex")
                        se = sp.tile([P, 1], f32, name="se")
                        nc.scalar.activation(
                            out=ex[:], in_=sub[:], func=AF.Exp, accum_out=se[:]
                        )
                        lse = sp.tile([P, 1], f32, name="lse")
                        nc.scalar.activation(out=lse[:], in_=se[:], func=AF.Ln)
                        ob = sp.tile([P, OUT], f32, name="ob")
                        nc.vector.tensor_scalar(
                            out=ob[:], in0=sub[:], scalar1=lse[:, 0:1], scalar2=None,
                            op0=OP.subtract,
                        )
                        nc.sync.dma_start(
                            out=out_d[bs : bs + rows, :], in_=ob[:rows, :]
                        )

    nc.compile()
    return nc


_CACHE = {}


def _get_nc(cfg_key, cfg, meta):
    key = (cfg_key, meta["KT"])
    if key not in _CACHE:
        _CACHE[key] = _build(cfg, meta)
    return _CACHE[key]


def kernel(x, edge_index, W0, b0, W1, b1, W2, b2):
    cfg = FULL_CFG
    in_maps, meta = _prep(x, edge_index, W0, b0, W1, b1, W2, b2, cfg)
    nc = _get_nc("full", cfg, meta)
    res = run_bass_kernel_spmd(nc, in_maps, core_ids=list(range(cfg["NCORES"])))
    out = np.concatenate([r["out"] for r in res.results], axis=0)
    return out


# revision 8
# speedup vs baseline: 1.0639x; 1.0639x over previous
"""BiGCN (3-layer binary GCN) on 8 TRN2 NeuronCores.

Strategy (per sharding hint): partition nodes (rows) across 8 cores; each
core owns N/8 destination nodes and all edges pointing at them. Per layer:
  z = sign(h) @ Wb            (dense matmul on own nodes; split-precision
                               fp16 hi+lo weights -> f32-accurate z)
  AllGather z -> full table   (cross-core exchange)
  agg[dst] = sum_e norm_e * z[src_e]   (dma_gather rows + precomputed
                                        one-hot T-matmul accumulation in
                                        PSUM, edges sorted by dst)
  h_next = agg + b            (fused into Sign / log_softmax epilogue)
Matmul and aggregation commute (aggregation is linear), so aggregating the
post-matmul activations minimizes exchange and gather width.
Batchnorm: sign(x - mu) — the variance cancels inside sign().

Host side does index/degree preprocessing (sharding, edge sorting, padding,
expansion of (dstloc, norm) pairs into dense selection matrices) and weight
binarization; all O(N*F) float work runs on device.
"""

import math

import numpy as np

import concourse.bass as bass
import concourse.mybir as mybir
from concourse import bacc
from concourse.tile import TileContext
from concourse.bass_utils import run_bass_kernel_spmd
from concourse.masks import make_identity

P = 128

FULL_CFG = dict(
    N=100000,
    IN=256,
    H=128,
    OUT=40,
    NCORES=8,
    NCH=4,     # src chunks for int16 gather indices
    TDT="f16",  # z-table / gather / T dtype: "f16" or "f32"
)


def _prep(x, edge_index, W0, b0, W1, b1, W2, b2, cfg):
    """Host-side sharding: degrees, edge bucketing by (core, dst-block,
    src-chunk), padding to a uniform tile structure, T-matrix expansion,
    weight binarization. Returns (in_maps, meta)."""
    N = cfg["N"]
    IN = cfg["IN"]
    H = cfg["H"]
    OUT = cfg["OUT"]
    C = cfg["NCORES"]
    NCH = cfg["NCH"]
    tnp = np.float16 if cfg["TDT"] == "f16" else np.float32
    SH = N // C
    NB = (SH + P - 1) // P
    CH = (N + NCH - 1) // NCH
    assert CH <= 32768, "gather indices are int16"
    assert N % C == 0

    x = np.asarray(x, dtype=np.float32)
    ei = np.asarray(edge_index)
    src = ei[0].astype(np.int64)
    dst = ei[1].astype(np.int64)

    deg = np.bincount(dst, minlength=N).astype(np.float64) + 1.0
    dinv = (deg ** -0.5).astype(np.float32)

    loop = np.arange(N, dtype=np.int64)
    srcA = np.concatenate([src, loop])
    dstA = np.concatenate([dst, loop])
    wA = (dinv[srcA] * dinv[dstA]).astype(np.float32)

    core = dstA // SH
    rel = dstA - core * SH
    blk = rel // P
    dloc = rel - blk * P
    q = srcA // CH
    il = (srcA - q * CH).astype(np.int16)

    key = (core * NB + blk) * NCH + q
    order = np.argsort(key, kind="stable")
    key_s = key[order]
    il_s = il[order]
    dloc_s = dloc[order]
    w_s = wA[order]

    NGRP = C * NB * NCH
    counts = np.bincount(key_s, minlength=NGRP)
    KT = int(math.ceil(counts.max() / P))
    CAP = KT * P

    starts = np.zeros(NGRP + 1, dtype=np.int64)
    np.cumsum(counts, out=starts[1:])
    pos = np.arange(len(key_s)) - starts[key_s]

    GRP = NB * NCH  # groups per core
    NT = GRP * KT   # tiles per core
    A_idx = np.zeros((C, GRP, CAP), dtype=np.int16)
    ccore = key_s // GRP
    cgrp = key_s - ccore * GRP
    A_idx[ccore, cgrp, pos] = il_s

    # dense selection matrices: T[c, g, s, p(edge), i(dst)] = w_e if edge
    # (g, s*128+p) targets local dst i, else 0.  Padded edges have w=0.
    T_all = np.zeros((C, GRP, KT, P, P), dtype=tnp)
    s_idx = pos // P
    p_idx = pos - s_idx * P
    T_all[ccore, cgrp, s_idx, p_idx, dloc_s] = w_s.astype(tnp)
    # device layout: [P(edge), NT*P] with tile t=(g*KT+s) at cols t*P:(t+1)*P
    T_dev = T_all.transpose(0, 3, 1, 2, 4).reshape(C, P, NT * P)

    # wrapped int16 index layout: edge j of a group -> [j%16, j//16],
    # replicated to 128 partitions (8 copies) for the 8 SWDGE Q7 cores.
    idx_w = (
        A_idx.reshape(C, GRP, CAP // 16, 16)
        .transpose(0, 3, 1, 2)
        .reshape(C, 16, GRP * (CAP // 16))
    )
    idx_all = np.tile(idx_w, (1, 8, 1))  # [C, 128, IDXCOLS]

    # weight binarization (XNOR-style) in split precision: hi + lo fp16
    def binw(W, pad_to=None):
        W = np.asarray(W, dtype=np.float32)
        alpha = np.abs(W).mean(axis=0, keepdims=True)
        Wb32 = np.sign(W) * alpha
        hi = Wb32.astype(np.float16)
        lo = (Wb32 - hi.astype(np.float32)).astype(np.float16)
        if pad_to is not None and hi.shape[1] < pad_to:
            hi = np.pad(hi, ((0, 0), (0, pad_to - hi.shape[1])))
            lo = np.pad(lo, ((0, 0), (0, pad_to - lo.shape[1])))
        return hi, lo

    Wb0, Wb0l = binw(W0)
    Wb1, Wb1l = binw(W1)
    Wb2, Wb2l = binw(W2, pad_to=P)
    bias0 = np.asarray(b0, np.float32).reshape(H, 1)
    bias1 = np.asarray(b1, np.float32).reshape(H, 1)
    bias2 = np.zeros((P, 1), np.float32)
    bias2[:OUT, 0] = np.asarray(b2, np.float32)

    in_maps = []
    for c in range(C):
        xs = np.ascontiguousarray(x[c * SH : (c + 1) * SH].T)  # [IN, SH] f32
        in_maps.append(
            dict(
                x_t=xs,
                Wb0=Wb0, Wb1=Wb1, Wb2=Wb2,
                Wb0l=Wb0l, Wb1l=Wb1l, Wb2l=Wb2l,
                bias0=bias0, bias1=bias1, bias2=bias2,
                idx_all=np.ascontiguousarray(idx_all[c]),
                T_dev=np.ascontiguousarray(T_dev[c]),
            )
        )
    meta = dict(KT=KT, SH=SH, NB=NB, CH=CH)
    return in_maps, meta


def _build(cfg, meta, dbg=False):
    """Build the SPMD Bass kernel (same program for all cores)."""
    N = cfg["N"]
    IN = cfg["IN"]
    H = cfg["H"]
    OUT = cfg["OUT"]
    C = cfg["NCORES"]
    NCH = cfg["NCH"]
    KT = meta["KT"]
    SH = meta["SH"]
    NB = meta["NB"]
    CH = meta["CH"]
    SHP = NB * P
    GRP = NB * NCH
    NT = GRP * KT
    IDXCOLS = GRP * KT * 8
    KC = (IN + P - 1) // P  # K-chunks for layer-0 matmul
    assert H == P
    f16 = mybir.dt.float16
    f32 = mybir.dt.float32
    tdt = f16 if cfg["TDT"] == "f16" else f32
    AF = mybir.ActivationFunctionType
    OP = mybir.AluOpType
    rg = [list(range(C))]

    nc = bacc.Bacc(None, target_bir_lowering=False, debug=False)

    x_t = nc.dram_tensor("x_t", [IN, SH], f32, kind="ExternalInput")
    Wb0 = nc.dram_tensor("Wb0", [IN, H], f16, kind="ExternalInput")
    Wb1 = nc.dram_tensor("Wb1", [H, H], f16, kind="ExternalInput")
    Wb2 = nc.dram_tensor("Wb2", [H, P], f16, kind="ExternalInput")
    Wb0l = nc.dram_tensor("Wb0l", [IN, H], f16, kind="ExternalInput")
    Wb1l = nc.dram_tensor("Wb1l", [H, H], f16, kind="ExternalInput")
    Wb2l = nc.dram_tensor("Wb2l", [H, P], f16, kind="ExternalInput")
    bias0 = nc.dram_tensor("bias0", [H, 1], f32, kind="ExternalInput")
    bias1 = nc.dram_tensor("bias1", [H, 1], f32, kind="ExternalInput")
    bias2 = nc.dram_tensor("bias2", [P, 1], f32, kind="ExternalInput")
    idx_all = nc.dram_tensor("idx_all", [P, IDXCOLS], mybir.dt.int16, kind="ExternalInput")
    T_d = nc.dram_tensor("T_dev", [P, NT * P], tdt, kind="ExternalInput")
    out_d = nc.dram_tensor("out", [SH, OUT], f32, kind="ExternalOutput")
    if dbg:
        dbg_z = nc.dram_tensor("dbg_z", [N, H], tdt, kind="ExternalOutput")
        dbg_s = nc.dram_tensor("dbg_s", [IN, SH], f16, kind="ExternalOutput")
        dbg_a = nc.dram_tensor("dbg_a", [H, SH], f32, kind="ExternalOutput")

    with TileContext(nc) as tc:
        with (
            tc.tile_pool(name="persist", bufs=1) as pp,
            tc.tile_pool(name="xin", bufs=2) as xp,
            tc.tile_pool(name="gpool", bufs=3) as gp,
            tc.tile_pool(name="tpool", bufs=3) as tp,
            tc.tile_pool(name="zpool", bufs=3) as zp,
            tc.tile_pool(name="spool", bufs=2) as sp,
            tc.tile_pool(name="psz", bufs=2, space="PSUM") as psz,
            tc.tile_pool(name="psa", bufs=2, space="PSUM") as psa,
            tc.tile_pool(name="pst", bufs=2, space="PSUM") as pst,
            tc.tile_pool(name="dram", bufs=1, space="DRAM") as dp,
        ):
            # ---- persistent SBUF state ----
            idx_sb = pp.tile([P, IDXCOLS], mybir.dt.int16)
            wb_sb = [pp.tile([P, H], f16, name=f"wb0_{k}") for k in range(KC)]
            wb1_sb = pp.tile([P, H], f16)
            wb2_sb = pp.tile([P, P], f16)
            wbl_sb = [pp.tile([P, H], f16, name=f"wb0l_{k}") for k in range(KC)]
            wb1l_sb = pp.tile([P, H], f16)
            wb2l_sb = pp.tile([P, P], f16)
            b0_sb = pp.tile([H, 1], f32)
            b1_sb = pp.tile([H, 1], f32)
            b2_sb = pp.tile([P, 1], f32)
            s_buf = [pp.tile([P, SHP], f16, name=f"sbuf_{k}") for k in range(KC)]
            nmu = [pp.tile([P, 1], f32, name=f"nmu_{k}") for k in range(KC)]

            nc.sync.dma_start(out=idx_sb[:], in_=idx_all[:])
            for k in range(KC):
                nc.sync.dma_start(out=wb_sb[k][:], in_=Wb0[k * P : (k + 1) * P, :])
                nc.sync.dma_start(out=wbl_sb[k][:], in_=Wb0l[k * P : (k + 1) * P, :])
            nc.sync.dma_start(out=wb1_sb[:], in_=Wb1[:])
            nc.sync.dma_start(out=wb2_sb[:], in_=Wb2[:])
            nc.sync.dma_start(out=wb1l_sb[:], in_=Wb1l[:])
            nc.sync.dma_start(out=wb2l_sb[:], in_=Wb2l[:])
            nc.sync.dma_start(out=b0_sb[:], in_=bias0[:])
            nc.sync.dma_start(out=b1_sb[:], in_=bias1[:])
            nc.sync.dma_start(out=b2_sb[:], in_=bias2[:])

            # ---- DRAM bounce buffers ----
            bn_in = dp.tile([IN, 1], f32)
            bn_out = dp.tile([IN, 1], f32, addr_space="Shared")
            z_in = [dp.tile([SH, H], tdt, name=f"z_in{k}") for k in range(3)]
            z_tab = [
                dp.tile([N, H], tdt, name=f"z_tab{k}", addr_space="Shared")
                for k in range(3)
            ]

            # x segments
            nseg = max(1, (SH + 2499) // 2500)
            seg = (SH + nseg - 1) // nseg
            segs = [(i * seg, min(SH, (i + 1) * seg)) for i in range(nseg)]

            # ---- BatchNorm: global column means via AllReduce ----
            for k in range(KC):
                stats = sp.tile([P, nseg], f32, name="stats")
                for t, (a, b) in enumerate(segs):
                    xt = xp.tile([P, seg], f32, name="xt")
                    nc.sync.dma_start(out=xt[:, : b - a], in_=x_t[k * P : (k + 1) * P, a:b])
                    nc.vector.reduce_sum(
                        stats[:, t : t + 1], xt[:, : b - a], axis=mybir.AxisListType.X
                    )
                s1 = sp.tile([P, 1], f32, name="s1")
                nc.vector.reduce_sum(s1[:], stats[:], axis=mybir.AxisListType.X)
                nc.sync.dma_start(out=bn_in[k * P : (k + 1) * P, :], in_=s1[:])
            nc.gpsimd.collective_compute(
                "AllReduce", OP.add, replica_groups=rg,
                ins=[bn_in[:].opt()], outs=[bn_out[:].opt()],
            )
            for k in range(KC):
                s1g = sp.tile([P, 1], f32, name="s1g")
                nc.sync.dma_start(out=s1g[:], in_=bn_out[k * P : (k + 1) * P, :])
                nc.vector.tensor_scalar(
                    out=nmu[k][:], in0=s1g[:], scalar1=-1.0 / N, scalar2=None,
                    op0=OP.mult,
                )

            # ---- layer-0 input signs: sign(x - mu) ----
            for k in range(KC):
                for a, b in segs:
                    xt = xp.tile([P, seg], f32, name="xt2")
                    nc.sync.dma_start(out=xt[:, : b - a], in_=x_t[k * P : (k + 1) * P, a:b])
                    nc.scalar.activation(
                        out=s_buf[k][:, a:b], in_=xt[:, : b - a], func=AF.Sign,
                        bias=nmu[k][:, 0:1],
                    )
                if SHP > SH:
                    nc.gpsimd.memset(s_buf[k][:, SH:SHP], 1.0)

            if dbg:
                for k in range(KC):
                    nc.sync.dma_start(out=dbg_s[k * P : (k + 1) * P, :], in_=s_buf[k][:, :SH])

            ident = pp.tile([P, P], f32)
            make_identity(nc, ident[:])

            rows_last = SH - (NB - 1) * P

            # ---- the three GCN layers ----
            for layer in range(3):
                if layer == 0:
                    wmats = [(k, wb_sb[k]) for k in range(KC)] + [
                        (k, wbl_sb[k]) for k in range(KC)
                    ]
                elif layer == 1:
                    wmats = [(0, wb1_sb), (0, wb1l_sb)]
                else:
                    wmats = [(0, wb2_sb), (0, wb2l_sb)]
                bias_sb = (b0_sb, b1_sb, b2_sb)[layer]
                zi, zt = z_in[layer], z_tab[layer]

                # z = sign(h) @ Wb for own nodes
                for nt in range(NB):
                    pz = psz.tile([P, H if layer < 2 else P], f32, name="pz")
                    for i, (k, wm) in enumerate(wmats):
                        nc.tensor.matmul(
                            pz[:], lhsT=s_buf[k][:, nt * P : (nt + 1) * P], rhs=wm[:],
                            start=(i == 0), stop=(i == len(wmats) - 1),
                        )
                    zr = zp.tile([P, H], tdt, name="zr")
                    nc.scalar.activation(out=zr[:], in_=pz[:, :H], func=AF.Copy)
                    rows = P if nt < NB - 1 else rows_last
                    nc.sync.dma_start(
                        out=zi[nt * P : nt * P + rows, :], in_=zr[:rows, :]
                    )

                nc.gpsimd.collective_compute(
                    "AllGather", OP.bypass, replica_groups=rg,
                    ins=[zi[:].opt()], outs=[zt[:].opt()],
                )
                if dbg and layer == 0:
                    nc.sync.dma_start(out=dbg_z[:], in_=zt[:])

                # aggregation over sorted edges, one dst-block at a time
                for b in range(NB):
                    pa = psa.tile([P, P], f32, name="pa")
                    for q in range(NCH):
                        g = b * NCH + q
                        G = gp.tile([P, KT, H], tdt, name="G")
                        nc.gpsimd.dma_gather(
                            G[:],
                            zt[q * CH : min(N, (q + 1) * CH), :],
                            idx_sb[:, g * KT * 8 : (g + 1) * KT * 8],
                            num_idxs=KT * P,
                            num_idxs_reg=KT * P,
                            elem_size=H,
                            elem_step=H,
                        )
                        Tg = tp.tile([P, KT * P], tdt, name="Tg")
                        nc.sync.dma_start(
                            out=Tg[:], in_=T_d[:, g * KT * P : (g + 1) * KT * P]
                        )
                        for s in range(KT):
                            nc.tensor.matmul(
                                pa[:], lhsT=G[:, s, :], rhs=Tg[:, s * P : (s + 1) * P],
                                start=(q == 0 and s == 0),
                                stop=(q == NCH - 1 and s == KT - 1),
                            )
                    bs = b * P
                    rows = P if b < NB - 1 else rows_last
                    if layer < 2:
                        if dbg and layer == 0:
                            hh = sp.tile([P, P], f32, name="hh")
                            nc.scalar.activation(
                                out=hh[:], in_=pa[:], func=AF.Identity, bias=bias_sb[:, 0:1]
                            )
                            nc.sync.dma_start(out=dbg_a[:, bs : bs + rows], in_=hh[:, :rows])
                        nc.scalar.activation(
                            out=s_buf[0][:, bs : bs + P], in_=pa[:], func=AF.Sign,
                            bias=bias_sb[:, 0:1],
                        )
                    else:
                        h2 = sp.tile([P, P], f32, name="h2")
                        nc.scalar.activation(
                            out=h2[:], in_=pa[:], func=AF.Identity, bias=bias_sb[:, 0:1]
                        )
                        ls_ps = pst.tile([P, P], f32, name="ls_ps")
                        nc.tensor.transpose(ls_ps[:], h2[:], ident[:])
                        ls = sp.tile([P, P], f32, name="ls")
                        nc.scalar.activation(out=ls[:], in_=ls_ps[:], func=AF.Copy)
                        m = sp.tile([P, 1], f32, name="m")
                        nc.vector.reduce_max(m[:], ls[:, :OUT], axis=mybir.AxisListType.X)
                        sub = sp.tile([P, OUT], f32, name="sub")
                        nc.vector.tensor_scalar(
                            out=sub[:], in0=ls[:, :OUT], scalar1=m[:, 0:1], scalar2=None,
                            op0=OP.subtract,
                        )
                        ex = sp.tile([P, OUT], f32, name="ex")
                        se = sp.tile([P, 1], f32, name="se")
                        nc.scalar.activation(
                            out=ex[:], in_=sub[:], func=AF.Exp, accum_out=se[:]
                        )
                        lse = sp.tile([P, 1], f32, name="lse")
                        nc.scalar.activation(out=lse[:], in_=se[:], func=AF.Ln)
                        ob = sp.tile([P, OUT], f32, name="ob")
                        nc.vector.tensor_scalar(
                            out=ob[:], in0=sub[:], scalar1=lse[:, 0:1], scalar2=None,
                            op0=OP.subtract,
                        )
                        nc.sync.dma_start(
                            out=out_d[bs : bs + rows, :], in_=ob[:rows, :]
                        )

    nc.compile()
    return nc


_CACHE = {}


def _get_nc(cfg_key, cfg, meta):
    key = (cfg_key, meta["KT"])
    if key not in _CACHE:
        _CACHE[key] = _build(cfg, meta)
    return _CACHE[key]


def kernel(x, edge_index, W0, b0, W1, b1, W2, b2):
    cfg = FULL_CFG
    in_maps, meta = _prep(x, edge_index, W0, b0, W1, b1, W2, b2, cfg)
    nc = _get_nc("full", cfg, meta)
    res = run_bass_kernel_spmd(nc, in_maps, core_ids=list(range(cfg["NCORES"])))
    out = np.concatenate([r["out"] for r in res.results], axis=0)
    return out
